# revision 23
# baseline (speedup 1.0000x reference)
"""CAM (channel-attention) kernel for Trainium2, 8-core batch-parallel.

Reference math per batch element b (x_b: [C=64, N=65536] fp32):
    q = x_b - mean(x_b, axis=1, keepdims=True)
    energy = (q @ q.T) / N                    # [64, 64]
    A = softmax(energy, axis=-1)
    out_b = gamma * (A @ q)                   # [64, N]

Strategy per core (one batch element per NeuronCore):
  - x is cast to bf16 on the HOST and uploaded as bf16 (halves both the
    host->device transfer and the on-device HBM read).  Verified numerically:
    single-bf16 everywhere gives max rel err ~2.4e-3 vs the fp32 reference
    (gate is 2e-2).
  - SBUF-resident stacked layout [p = h*64 + c, n] (halves of N side by side)
    so every DMA row is contiguous and the PE sees 128 full partitions.
  - Pass 1 (Gram): PE transposes [128,128] subblocks, staging tiles carry an
    extra all-ones column, one [128,129] matmul per subblock accumulates
    per-half Grams (diag blocks) + channel sums (col 128) in a single PSUM
    accumulator.  out = A'@x - (A'@mean) so the mean never touches x.
  - Softmax on [64,64]; gamma folded in; A' cast to bf16 into a
    block-diagonal [128,128] lhsT so pass 2 needs ONE matmul per 512 cols.
  - Pass 2: out = A'@x + bias via 64 matmuls; PSUM->SBUF epilogue adds the
    bias and casts to bf16 (DVE/ACT alternating); bf16 streams out.
  - DMA: the gpsimd SWDGE queue spreads descriptors over all 16 DMA engines
    (~175 GB/s) but queued transfers complete together near the end of the
    stream; the HWDGE queues (sync/scalar) are slow (~25 GB/s each, shared
    engine pair) but complete per-chunk promptly.  Bulk traffic rides SWDGE
    deep-queued, with a few mid-stream chunks on the HWDGE queues so the PE
    has work before the SWDGE bulk lands.
"""

import sys

if "/opt/trn_rl_repo" not in sys.path:
    sys.path.insert(0, "/opt/trn_rl_repo")

import numpy as np

import concourse.bass as bass
import concourse.tile as tile
from concourse import bacc, mybir
from concourse.bass_utils import run_bass_kernel_spmd
from concourse.masks import make_identity

F32 = mybir.dt.float32
BF16 = mybir.dt.bfloat16
I16 = mybir.dt.int16
ACT_F = mybir.ActivationFunctionType
ALU = mybir.AluOpType

B, C, H, W = 8, 64, 256, 256
N = H * W          # 65536
HALF = N // 2      # 32768 columns per partition-half
CH = 2048          # bulk chunk columns (stacked layout) -> 512 KiB bf16
NCHUNK = HALF // CH  # 16
RPC = N // CH      # DRAM rows per channel in [rows, CH] view (16)
NPRE = 4           # input chunks issued ahead (SWDGE descriptor window)
SUB = 128          # transpose subblock columns
TB = 4             # transposes per PSUM batch ([128, 512])
P2CH = 512         # pass-2 chunk columns (one PSUM bank)
OG = 8             # pass-2 chunks per output stage ([128, 4096] -> 1 MiB)
NQ = 4             # SWDGE queues


def build(sim_safe=False):
    nc = bacc.Bacc(None, target_bir_lowering=False)
    x_d = nc.dram_tensor("x", [C, N], BF16, kind="ExternalInput")
    g_d = nc.dram_tensor("gamma", [1, 1], F32, kind="ExternalInput")
    out_d = nc.dram_tensor("out", [C, N], BF16, kind="ExternalOutput")

    # 3D views ordered (h, c, n): stream order matches the stacked SBUF
    # layout [p = h*64+c, n]; one DMA covers both partition halves.
    x_v = x_d.ap().rearrange("c (h n) -> h c n", h=2)
    out_v = out_d.ap().rearrange("c (h n) -> h c n", h=2)

    with tile.TileContext(nc) as tc, \
         tc.tile_pool(name="constp", bufs=1) as constp, \
         tc.tile_pool(name="smalls", bufs=2) as smalls:
        # resident x (bf16), stacked layout [128, HALF]
        xb_sb = constp.tile([128, HALF], BF16)

        def in_dma(k):
            nc.gpsimd.dma_start(
                out=xb_sb[:, k * CH : (k + 1) * CH],
                in_=x_v[:, :, k * CH : (k + 1) * CH],
            )

        # the first NPRE chunk loads go out before anything else so the
        # SWDGE queue starts moving data during const setup
        for k in range(NPRE):
            in_dma(k)

        # ---------------- constants / persistent tiles ----------------
        ident128 = constp.tile([128, 128], BF16)
        make_identity(nc, ident128)
        ident64f = constp.tile([64, 64], F32)
        make_identity(nc, ident64f)
        ones_row = constp.tile([1, 128], F32)
        nc.gpsimd.memset(ones_row, 1.0)

        g_sb = constp.tile([1, 1], F32)
        nc.sync.dma_start(out=g_sb, in_=g_d.ap())

        # [1,1] probe target for DMA pacing reads
        probe_sb = constp.tile([1, 1], BF16)

        # staging ring for transposed tiles: TB groups of 129 columns,
        # group = [xT(128) | 1]; col 128 preset 1.0 (never overwritten)
        NRING = 3
        T_st = []
        for i in range(NRING):
            t = constp.tile([128, 129 * TB], BF16, name=f"T_st{i}")
            T_st.append(t)
            nc.gpsimd.memset(
                t.rearrange("p (g w) -> p g w", w=129)[:, :, 128:129], 1.0
            )

        # block-diagonal [[A'^T, 0], [0, A'^T]] lhsT for pass 2
        AT2 = constp.tile([128, 128], BF16)
        nc.gpsimd.memset(AT2, 0.0)
        negb = constp.tile([128, 1], F32)
        g_bcast = constp.tile([128, 1], F32)
        u2_bf = constp.tile([128, 1], BF16)

        with (
            tc.tile_pool(name="psG", bufs=1, space="PSUM") as psG,
            tc.tile_pool(name="psT", bufs=2, space="PSUM") as psT,
            tc.tile_pool(name="psS", bufs=2, space="PSUM") as psS,
        ):
            # fused Gram accumulator: [0:64,0:64]+[64:128,64:128] = raw Gram
            # blocks per half; col 128 = per-stacked-channel sums
            G2 = psG.tile([128, 129], F32, tag="g2")

            # PE warmup: absorb gpsimd const deps into the PE clock.
            warm_ps = psS.tile([128, 128], BF16, tag="small")
            nc.tensor.matmul(warm_ps, ident128, ident128, is_transpose=True)
            # preload exp activation table early (off the critical path)
            exp_scr = smalls.tile([1, 1], F32, tag="escr")
            nc.scalar.activation(exp_scr, ones_row[0:1, 0:1], ACT_F.Exp)

            # gamma broadcast to all 128 partitions (K=1 matmul trick)
            gb_ps = psS.tile([128, 1], F32, tag="small")
            nc.tensor.matmul(gb_ps, ones_row, g_sb, start=True, stop=True)
            nc.vector.tensor_copy(g_bcast, gb_ps)

            # ---------------- phase 1: load + transpose + Gram ----------------
            # Per chunk: CH/SUB transposes in batches of TB; each batch: TB
            # transposes -> one strided PSUM->staging copy (DVE/ACT
            # alternating) -> TB [128,129] Gram matmuls.  Batches are
            # software-pipelined: batch b's Gram matmuls are emitted after
            # batch b+1's transposes so the PE never waits on the staging
            # copy.
            total_batches = NCHUNK * (CH // (SUB * TB))  # 16 * 4 = 64
            nbpc = CH // (SUB * TB)                      # batches per chunk

            def emit_gram_mms(b):
                st = T_st[b % NRING]
                st_v = st.rearrange("p (g w) -> p g w", w=129)
                for g in range(TB):
                    nc.tensor.matmul(
                        G2,
                        st_v[:, g, 0:128],
                        st_v[:, g, 0:129],
                        start=(b == 0 and g == 0),
                        stop=(b == total_batches - 1 and g == TB - 1),
                        skip_group_check=True,
                    )

            for k in range(NCHUNK):
                xslice = xb_sb[:, k * CH : (k + 1) * CH]
                # DGE pacing: before issuing chunk k's load, make the (idle)
                # gpsimd engine block until chunk k-NPRE has fully landed.
                # This bounds the SWDGE descriptor window so chunk
                # completions stagger instead of all arriving at the end of
                # the whole input phase (measured pathology).
                if k >= NPRE:
                    nc.gpsimd.tensor_copy(
                        probe_sb,
                        xb_sb[0:1, (k - NPRE) * CH : (k - NPRE) * CH + 1],
                    )
                    in_dma(k)
                for bb in range(nbpc):
                    b = k * nbpc + bb
                    ph = psT.tile([128, SUB * TB], BF16, tag="psTh")
                    for jj in range(TB):
                        src = slice((bb * TB + jj) * SUB, (bb * TB + jj + 1) * SUB)
                        nc.tensor.matmul(
                            ph[:, jj * SUB : (jj + 1) * SUB],
                            xslice[:, src],
                            ident128,
                            is_transpose=True,
                        )
                    st = T_st[b % NRING]
                    st_v = st.rearrange("p (g w) -> p g w", w=129)
                    ph_v = ph.rearrange("p (g w) -> p g w", w=SUB)
                    if b % 2 == 0:
                        nc.vector.tensor_copy(st_v[:, :, 0:128], ph_v)
                    else:
                        nc.scalar.activation(st_v[:, :, 0:128], ph_v, ACT_F.Copy)
                    if b > 0:
                        emit_gram_mms(b - 1)
            emit_gram_mms(total_batches - 1)

            # ------------- phase 1.5: energy, softmax, A', bias -----------
            Gs = smalls.tile([128, 129], F32, tag="gs")
            nc.scalar.activation(Gs, G2, ACT_F.Copy)
            # bring the half-1 diag block + sums down to partitions 0:64
            Gtmp = smalls.tile([64, 65], F32, tag="gtmp")
            nc.sync.dma_start(out=Gtmp, in_=Gs[64:128, 64:129])
            Gsum = smalls.tile([64, 64], F32, tag="gsum")
            nc.vector.tensor_add(Gsum, Gs[0:64, 0:64], Gtmp[:, 0:64])
            s_col = smalls.tile([64, 1], F32, tag="scol")
            nc.vector.tensor_add(s_col, Gs[0:64, 128:129], Gtmp[:, 64:65])
            u_col = smalls.tile([64, 1], F32, tag="ucol")
            nc.vector.tensor_scalar_mul(u_col, s_col, 1.0 / N)

            # outer product uu^T via [64,1] -> [1,64] transpose + K=1 matmul
            urow_ps = psS.tile([1, 64], F32, tag="small")
            nc.tensor.matmul(urow_ps, u_col, ident64f, is_transpose=True)
            urow = smalls.tile([1, 64], F32, tag="urow")
            nc.vector.tensor_copy(urow, urow_ps)
            uuT_ps = psS.tile([64, 64], F32, tag="small")
            nc.tensor.matmul(uuT_ps, urow, urow, start=True, stop=True)

            # E = Gsum/N - uu^T
            E_sb = smalls.tile([64, 64], F32, tag="esb")
            nc.vector.tensor_scalar_mul(E_sb, Gsum, 1.0 / N)
            nc.vector.tensor_sub(E_sb, E_sb, uuT_ps)

            # row softmax; fold gamma into A'
            negm = smalls.tile([64, 1], F32, tag="negm")
            nc.vector.tensor_reduce(
                negm, E_sb, axis=mybir.AxisListType.X, op=ALU.max, negate=True
            )
            P_sb = smalls.tile([64, 64], F32, tag="psb")
            nc.scalar.activation(P_sb, E_sb, ACT_F.Exp, bias=negm, scale=1.0)
            z = smalls.tile([64, 1], F32, tag="z")
            nc.vector.reduce_sum(z, P_sb, axis=mybir.AxisListType.X)
            rz = smalls.tile([64, 1], F32, tag="rz")
            nc.vector.reciprocal(rz, z)
            rg = smalls.tile([64, 1], F32, tag="rg")
            nc.vector.tensor_mul(rg, rz, g_bcast[0:64, :])
            A2 = smalls.tile([64, 128], F32, tag="a2")
            nc.vector.tensor_scalar_mul(A2[:, 0:64], P_sb, rg)
            nc.vector.tensor_copy(A2[:, 64:128], A2[:, 0:64])

            # AT2 block-diag: transpose [A'|A'] -> [128,64] stacked, then
            # bf16-cast the two halves into the diagonal blocks
            AT_ps = psS.tile([128, 64], F32, tag="small")
            nc.tensor.matmul(AT_ps, A2, ident64f, is_transpose=True)
            nc.vector.tensor_copy(AT2[0:64, 0:64], AT_ps[0:64, :])
            nc.scalar.activation(AT2[64:128, 64:128], AT_ps[64:128, :], ACT_F.Copy)

            # bias: negb = -(A' @ u) on all 128 partitions
            nc.vector.tensor_copy(u2_bf[0:64, :], u_col)
            nc.sync.dma_start(out=u2_bf[64:128, :], in_=u2_bf[0:64, :])
            b1_ps = psS.tile([128, 1], F32, tag="small")
            nc.tensor.matmul(b1_ps, AT2, u2_bf, start=True, stop=True)
            nc.vector.tensor_scalar_mul(negb, b1_ps, -1.0)
            # touch negb on ACT too so the epilogue's ACT instrs have their
            # wait absorbed off the critical path
            scr_a = smalls.tile([128, 1], F32, tag="scra")
            nc.scalar.activation(scr_a, negb, ACT_F.Copy)

        # ---------------- phase 2: out = A'@x + negb ----------------
        with (
            tc.tile_pool(name="ps2", bufs=8, space="PSUM") as ps2,
            tc.tile_pool(name="ostage", bufs=8) as ostage,
        ):
            n_ch = HALF // P2CH  # 64
            stage = None
            for p in range(n_ch):
                if p % OG == 0:
                    stage = ostage.tile([128, OG * P2CH], BF16, tag="ost")
                pso = ps2.tile([128, P2CH], F32, tag="pso")
                cols = slice(p * P2CH, (p + 1) * P2CH)
                nc.tensor.matmul(
                    pso, AT2, xb_sb[:, cols], start=True, stop=True,
                )
                dst = stage[:, (p % OG) * P2CH : (p % OG + 1) * P2CH]
                if p % 2 == 0:
                    nc.vector.tensor_scalar_add(dst, pso, negb)
                else:
                    # CoreSim lacks Prelu; Relu with bias has identical cost
                    # and serves for sim-timing runs.
                    f = ACT_F.Relu if sim_safe else ACT_F.Prelu
                    nc.scalar.activation(
                        dst, pso, f, bias=negb, scale=1.0, alpha=1.0
                    )
                if p % OG == OG - 1:
                    q = p // OG
                    dst = out_v[:, :, q * OG * P2CH : (q + 1) * OG * P2CH]
                    nc.gpsimd.dma_start(out=dst, in_=stage)

    nc.finalize()
    return nc


_CACHED = None


def _get_nc():
    global _CACHED
    if _CACHED is None:
        _CACHED = build()
    return _CACHED


def _to_bf16(a):
    import ml_dtypes

    return np.asarray(a).astype(ml_dtypes.bfloat16)


def _in_maps(x: np.ndarray, gamma: np.ndarray) -> list:
    xb = _to_bf16(np.ascontiguousarray(x)).reshape(B, C, N)
    g = np.asarray(gamma, dtype=np.float32).reshape(1, 1)
    return [{"x": xb[i], "gamma": g} for i in range(B)]


def kernel(x: np.ndarray, gamma: np.ndarray) -> np.ndarray:
    assert x.shape == (B, C, H, W), x.shape
    nc = _get_nc()
    res = run_bass_kernel_spmd(nc, _in_maps(x, gamma), core_ids=list(range(B)))
    out = np.stack([res.results[i]["out"] for i in range(B)])
    return out.reshape(B, C, H, W).astype(np.float32)


if __name__ == "__main__":
    rng = np.random.default_rng(0)
    x = rng.standard_normal((B, C, H, W), dtype=np.float32)
    gamma = rng.standard_normal((1,), dtype=np.float32)
    y = kernel(x, gamma)
    print("ran ok", y.shape, y.dtype)


# revision 24
# speedup vs baseline: 1.0982x; 1.0982x over previous
"""CAM (channel-attention) kernel for Trainium2, 8-core batch-parallel.

Reference math per batch element b (x_b: [C=64, N=65536] fp32):
    q = x_b - mean(x_b, axis=1, keepdims=True)
    energy = (q @ q.T) / N                    # [64, 64]
    A = softmax(energy, axis=-1)
    out_b = gamma * (A @ q)                   # [64, N]

Strategy per core (one batch element per NeuronCore):
  - x is cast to bf16 on the HOST and uploaded as bf16 (halves both the
    host->device transfer and the on-device HBM read).  Verified numerically:
    single-bf16 everywhere gives max rel err ~2.4e-3 vs the fp32 reference
    (gate is 2e-2).
  - SBUF-resident stacked layout [p = h*64 + c, n] (halves of N side by side)
    so every DMA row is contiguous and the PE sees 128 full partitions.
  - Pass 1 (Gram): PE transposes [128,128] subblocks, staging tiles carry an
    extra all-ones column, one [128,129] matmul per subblock accumulates
    per-half Grams (diag blocks) + channel sums (col 128) in a single PSUM
    accumulator.  out = A'@x - (A'@mean) so the mean never touches x.
  - Softmax on [64,64]; gamma folded in; A' cast to bf16 into a
    block-diagonal [128,128] lhsT so pass 2 needs ONE matmul per 512 cols.
  - Pass 2: out = A'@x + bias via 64 matmuls; PSUM->SBUF epilogue adds the
    bias and casts to bf16 (DVE/ACT alternating); bf16 streams out.
  - DMA: the gpsimd SWDGE queue spreads descriptors over all 16 DMA engines
    (~175 GB/s) but queued transfers complete together near the end of the
    stream; the HWDGE queues (sync/scalar) are slow (~25 GB/s each, shared
    engine pair) but complete per-chunk promptly.  Bulk traffic rides SWDGE
    deep-queued, with a few mid-stream chunks on the HWDGE queues so the PE
    has work before the SWDGE bulk lands.
"""

import sys

if "/opt/trn_rl_repo" not in sys.path:
    sys.path.insert(0, "/opt/trn_rl_repo")

import numpy as np

import concourse.bass as bass
import concourse.tile as tile
from concourse import bacc, mybir
from concourse.bass_utils import run_bass_kernel_spmd
from concourse.masks import make_identity

F32 = mybir.dt.float32
BF16 = mybir.dt.bfloat16
I16 = mybir.dt.int16
ACT_F = mybir.ActivationFunctionType
ALU = mybir.AluOpType

B, C, H, W = 8, 64, 256, 256
N = H * W          # 65536
HALF = N // 2      # 32768 columns per partition-half
CH = 2048          # bulk chunk columns (stacked layout) -> 512 KiB bf16
NCHUNK = HALF // CH  # 16
RPC = N // CH      # DRAM rows per channel in [rows, CH] view (16)
NPRE = 4           # input chunks issued ahead (SWDGE descriptor window)
SUB = 128          # transpose subblock columns
TB = 4             # transposes per PSUM batch ([128, 512])
P2CH = 512         # pass-2 chunk columns (one PSUM bank)
OG = 8             # pass-2 chunks per output stage ([128, 4096] -> 1 MiB)
NQ = 4             # SWDGE queues


def build(sim_safe=False):
    nc = bacc.Bacc(None, target_bir_lowering=False)
    x_d = nc.dram_tensor("x", [C, N], BF16, kind="ExternalInput")
    g_d = nc.dram_tensor("gamma", [1, 1], F32, kind="ExternalInput")
    out_d = nc.dram_tensor("out", [C, N], BF16, kind="ExternalOutput")

    # 3D views ordered (h, c, n): stream order matches the stacked SBUF
    # layout [p = h*64+c, n]; one DMA covers both partition halves.
    x_v = x_d.ap().rearrange("c (h n) -> h c n", h=2)
    out_v = out_d.ap().rearrange("c (h n) -> h c n", h=2)

    with tile.TileContext(nc) as tc, \
         tc.tile_pool(name="constp", bufs=1) as constp, \
         tc.tile_pool(name="smalls", bufs=2) as smalls:
        # resident x (bf16), stacked layout [128, HALF]
        xb_sb = constp.tile([128, HALF], BF16)

        def in_dma(k):
            nc.gpsimd.dma_start(
                out=xb_sb[:, k * CH : (k + 1) * CH],
                in_=x_v[:, :, k * CH : (k + 1) * CH],
            )

        # First chunk load goes out before anything else; the remaining
        # upfront loads are spaced with ~4us gpsimd delay memsets so chunk
        # completions stagger (the SWDGE queue serves all queued transfers
        # round-robin, so same-time issues complete together and leave the
        # PE bursty).
        in_dma(0)

        # ---------------- constants / persistent tiles ----------------
        ident128 = constp.tile([128, 128], BF16)
        make_identity(nc, ident128)
        ident64f = constp.tile([64, 64], F32)
        make_identity(nc, ident64f)
        ones_row = constp.tile([1, 128], F32)
        nc.gpsimd.memset(ones_row, 1.0)

        delay_sb = constp.tile([128, 1536], BF16)
        for k in range(1, NPRE):
            nc.gpsimd.memset(delay_sb, 0.0)
            in_dma(k)

        g_sb = constp.tile([1, 1], F32)
        nc.sync.dma_start(out=g_sb, in_=g_d.ap())

        # [1,1] probe target for DMA pacing reads
        probe_sb = constp.tile([1, 1], BF16)

        # staging ring for transposed tiles: TB groups of 129 columns,
        # group = [xT(128) | 1]; col 128 preset 1.0 (never overwritten)
        NRING = 3
        T_st = []
        for i in range(NRING):
            t = constp.tile([128, 129 * TB], BF16, name=f"T_st{i}")
            T_st.append(t)
            nc.gpsimd.memset(
                t.rearrange("p (g w) -> p g w", w=129)[:, :, 128:129], 1.0
            )

        # block-diagonal [[A'^T, 0], [0, A'^T]] lhsT for pass 2
        AT2 = constp.tile([128, 128], BF16)
        nc.gpsimd.memset(AT2, 0.0)
        negb = constp.tile([128, 1], F32)
        g_bcast = constp.tile([128, 1], F32)
        u2_bf = constp.tile([128, 1], BF16)

        with (
            tc.tile_pool(name="psG", bufs=1, space="PSUM") as psG,
            tc.tile_pool(name="psT", bufs=2, space="PSUM") as psT,
            tc.tile_pool(name="psS", bufs=2, space="PSUM") as psS,
        ):
            # fused Gram accumulator: [0:64,0:64]+[64:128,64:128] = raw Gram
            # blocks per half; col 128 = per-stacked-channel sums
            G2 = psG.tile([128, 129], F32, tag="g2")

            # PE warmup: absorb gpsimd const deps into the PE clock.
            warm_ps = psS.tile([128, 128], BF16, tag="small")
            nc.tensor.matmul(warm_ps, ident128, ident128, is_transpose=True)
            # preload exp activation table early (off the critical path)
            exp_scr = smalls.tile([1, 1], F32, tag="escr")
            nc.scalar.activation(exp_scr, ones_row[0:1, 0:1], ACT_F.Exp)

            # gamma broadcast to all 128 partitions (K=1 matmul trick)
            gb_ps = psS.tile([128, 1], F32, tag="small")
            nc.tensor.matmul(gb_ps, ones_row, g_sb, start=True, stop=True)
            nc.vector.tensor_copy(g_bcast, gb_ps)

            # ---------------- phase 1: load + transpose + Gram ----------------
            # Per chunk: CH/SUB transposes in batches of TB; each batch: TB
            # transposes -> one strided PSUM->staging copy (DVE/ACT
            # alternating) -> TB [128,129] Gram matmuls.  Batches are
            # software-pipelined: batch b's Gram matmuls are emitted after
            # batch b+1's transposes so the PE never waits on the staging
            # copy.
            total_batches = NCHUNK * (CH // (SUB * TB))  # 16 * 4 = 64
            nbpc = CH // (SUB * TB)                      # batches per chunk

            def emit_gram_mms(b):
                st = T_st[b % NRING]
                st_v = st.rearrange("p (g w) -> p g w", w=129)
                for g in range(TB):
                    nc.tensor.matmul(
                        G2,
                        st_v[:, g, 0:128],
                        st_v[:, g, 0:129],
                        start=(b == 0 and g == 0),
                        stop=(b == total_batches - 1 and g == TB - 1),
                        skip_group_check=True,
                    )

            for k in range(NCHUNK):
                xslice = xb_sb[:, k * CH : (k + 1) * CH]
                # DGE pacing: before issuing chunk k's load, make the (idle)
                # gpsimd engine block until chunk k-NPRE has fully landed.
                # This bounds the SWDGE descriptor window so chunk
                # completions stagger instead of all arriving at the end of
                # the whole input phase (measured pathology).
                if k >= NPRE:
                    nc.gpsimd.tensor_copy(
                        probe_sb,
                        xb_sb[0:1, (k - NPRE) * CH : (k - NPRE) * CH + 1],
                    )
                    in_dma(k)
                for bb in range(nbpc):
                    b = k * nbpc + bb
                    ph = psT.tile([128, SUB * TB], BF16, tag="psTh")
                    for jj in range(TB):
                        src = slice((bb * TB + jj) * SUB, (bb * TB + jj + 1) * SUB)
                        nc.tensor.matmul(
                            ph[:, jj * SUB : (jj + 1) * SUB],
                            xslice[:, src],
                            ident128,
                            is_transpose=True,
                        )
                    st = T_st[b % NRING]
                    st_v = st.rearrange("p (g w) -> p g w", w=129)
                    ph_v = ph.rearrange("p (g w) -> p g w", w=SUB)
                    if b % 2 == 0:
                        nc.vector.tensor_copy(st_v[:, :, 0:128], ph_v)
                    else:
                        nc.scalar.activation(st_v[:, :, 0:128], ph_v, ACT_F.Copy)
                    if b > 0:
                        emit_gram_mms(b - 1)
            emit_gram_mms(total_batches - 1)

            # ------------- phase 1.5: energy, softmax, A', bias -----------
            Gs = smalls.tile([128, 129], F32, tag="gs")
            nc.scalar.activation(Gs, G2, ACT_F.Copy)
            # bring the half-1 diag block + sums down to partitions 0:64
            Gtmp = smalls.tile([64, 65], F32, tag="gtmp")
            nc.sync.dma_start(out=Gtmp, in_=Gs[64:128, 64:129])
            Gsum = smalls.tile([64, 64], F32, tag="gsum")
            nc.vector.tensor_add(Gsum, Gs[0:64, 0:64], Gtmp[:, 0:64])
            s_col = smalls.tile([64, 1], F32, tag="scol")
            nc.vector.tensor_add(s_col, Gs[0:64, 128:129], Gtmp[:, 64:65])
            u_col = smalls.tile([64, 1], F32, tag="ucol")
            nc.vector.tensor_scalar_mul(u_col, s_col, 1.0 / N)

            # outer product uu^T via [64,1] -> [1,64] transpose + K=1 matmul
            urow_ps = psS.tile([1, 64], F32, tag="small")
            nc.tensor.matmul(urow_ps, u_col, ident64f, is_transpose=True)
            urow = smalls.tile([1, 64], F32, tag="urow")
            nc.vector.tensor_copy(urow, urow_ps)
            uuT_ps = psS.tile([64, 64], F32, tag="small")
            nc.tensor.matmul(uuT_ps, urow, urow, start=True, stop=True)

            # E = Gsum/N - uu^T
            E_sb = smalls.tile([64, 64], F32, tag="esb")
            nc.vector.tensor_scalar_mul(E_sb, Gsum, 1.0 / N)
            nc.vector.tensor_sub(E_sb, E_sb, uuT_ps)

            # row softmax; fold gamma into A'
            negm = smalls.tile([64, 1], F32, tag="negm")
            nc.vector.tensor_reduce(
                negm, E_sb, axis=mybir.AxisListType.X, op=ALU.max, negate=True
            )
            P_sb = smalls.tile([64, 64], F32, tag="psb")
            nc.scalar.activation(P_sb, E_sb, ACT_F.Exp, bias=negm, scale=1.0)
            z = smalls.tile([64, 1], F32, tag="z")
            nc.vector.reduce_sum(z, P_sb, axis=mybir.AxisListType.X)
            rz = smalls.tile([64, 1], F32, tag="rz")
            nc.vector.reciprocal(rz, z)
            rg = smalls.tile([64, 1], F32, tag="rg")
            nc.vector.tensor_mul(rg, rz, g_bcast[0:64, :])
            A2 = smalls.tile([64, 128], F32, tag="a2")
            nc.vector.tensor_scalar_mul(A2[:, 0:64], P_sb, rg)
            nc.vector.tensor_copy(A2[:, 64:128], A2[:, 0:64])

            # AT2 block-diag: transpose [A'|A'] -> [128,64] stacked, then
            # bf16-cast the two halves into the diagonal blocks
            AT_ps = psS.tile([128, 64], F32, tag="small")
            nc.tensor.matmul(AT_ps, A2, ident64f, is_transpose=True)
            nc.vector.tensor_copy(AT2[0:64, 0:64], AT_ps[0:64, :])
            nc.scalar.activation(AT2[64:128, 64:128], AT_ps[64:128, :], ACT_F.Copy)

            # bias: negb = -(A' @ u) on all 128 partitions
            nc.vector.tensor_copy(u2_bf[0:64, :], u_col)
            nc.sync.dma_start(out=u2_bf[64:128, :], in_=u2_bf[0:64, :])
            b1_ps = psS.tile([128, 1], F32, tag="small")
            nc.tensor.matmul(b1_ps, AT2, u2_bf, start=True, stop=True)
            nc.vector.tensor_scalar_mul(negb, b1_ps, -1.0)
            # touch negb on ACT too so the epilogue's ACT instrs have their
            # wait absorbed off the critical path
            scr_a = smalls.tile([128, 1], F32, tag="scra")
            nc.scalar.activation(scr_a, negb, ACT_F.Copy)

        # ---------------- phase 2: out = A'@x + negb ----------------
        with (
            tc.tile_pool(name="ps2", bufs=8, space="PSUM") as ps2,
            tc.tile_pool(name="ostage", bufs=8) as ostage,
        ):
            n_ch = HALF // P2CH  # 64
            stage = None
            for p in range(n_ch):
                if p % OG == 0:
                    stage = ostage.tile([128, OG * P2CH], BF16, tag="ost")
                pso = ps2.tile([128, P2CH], F32, tag="pso")
                cols = slice(p * P2CH, (p + 1) * P2CH)
                nc.tensor.matmul(
                    pso, AT2, xb_sb[:, cols], start=True, stop=True,
                )
                dst = stage[:, (p % OG) * P2CH : (p % OG + 1) * P2CH]
                if p % 2 == 0:
                    nc.vector.tensor_scalar_add(dst, pso, negb)
                else:
                    # CoreSim lacks Prelu; Relu with bias has identical cost
                    # and serves for sim-timing runs.
                    f = ACT_F.Relu if sim_safe else ACT_F.Prelu
                    nc.scalar.activation(
                        dst, pso, f, bias=negb, scale=1.0, alpha=1.0
                    )
                if p % OG == OG - 1:
                    q = p // OG
                    dst = out_v[:, :, q * OG * P2CH : (q + 1) * OG * P2CH]
                    nc.gpsimd.dma_start(out=dst, in_=stage)

    nc.finalize()
    return nc


_CACHED = None


def _get_nc():
    global _CACHED
    if _CACHED is None:
        _CACHED = build()
    return _CACHED


def _to_bf16(a):
    import ml_dtypes

    return np.asarray(a).astype(ml_dtypes.bfloat16)


def _in_maps(x: np.ndarray, gamma: np.ndarray) -> list:
    xb = _to_bf16(np.ascontiguousarray(x)).reshape(B, C, N)
    g = np.asarray(gamma, dtype=np.float32).reshape(1, 1)
    return [{"x": xb[i], "gamma": g} for i in range(B)]


def kernel(x: np.ndarray, gamma: np.ndarray) -> np.ndarray:
    assert x.shape == (B, C, H, W), x.shape
    nc = _get_nc()
    res = run_bass_kernel_spmd(nc, _in_maps(x, gamma), core_ids=list(range(B)))
    out = np.stack([res.results[i]["out"] for i in range(B)])
    return out.reshape(B, C, H, W).astype(np.float32)


if __name__ == "__main__":
    rng = np.random.default_rng(0)
    x = rng.standard_normal((B, C, H, W), dtype=np.float32)
    gamma = rng.standard_normal((1,), dtype=np.float32)
    y = kernel(x, gamma)
    print("ran ok", y.shape, y.dtype)


# revision 25
# speedup vs baseline: 1.1273x; 1.0265x over previous
"""CAM (channel-attention) kernel for Trainium2, 8-core batch-parallel.

Reference math per batch element b (x_b: [C=64, N=65536] fp32):
    q = x_b - mean(x_b, axis=1, keepdims=True)
    energy = (q @ q.T) / N                    # [64, 64]
    A = softmax(energy, axis=-1)
    out_b = gamma * (A @ q)                   # [64, N]

Strategy per core (one batch element per NeuronCore):
  - x is cast to bf16 on the HOST and uploaded as bf16 (halves both the
    host->device transfer and the on-device HBM read).  Verified numerically:
    single-bf16 everywhere gives max rel err ~2.4e-3 vs the fp32 reference
    (gate is 2e-2).
  - SBUF-resident stacked layout [p = h*64 + c, n] (halves of N side by side)
    so every DMA row is contiguous and the PE sees 128 full partitions.
  - Pass 1 (Gram): PE transposes [128,128] subblocks, staging tiles carry an
    extra all-ones column, one [128,129] matmul per subblock accumulates
    per-half Grams (diag blocks) + channel sums (col 128) in a single PSUM
    accumulator.  out = A'@x - (A'@mean) so the mean never touches x.
  - Softmax on [64,64]; gamma folded in; A' cast to bf16 into a
    block-diagonal [128,128] lhsT so pass 2 needs ONE matmul per 512 cols.
  - Pass 2: out = A'@x + bias via 64 matmuls; PSUM->SBUF epilogue adds the
    bias and casts to bf16 (DVE/ACT alternating); bf16 streams out.
  - DMA: the gpsimd SWDGE queue spreads descriptors over all 16 DMA engines
    (~175 GB/s) but queued transfers complete together near the end of the
    stream; the HWDGE queues (sync/scalar) are slow (~25 GB/s each, shared
    engine pair) but complete per-chunk promptly.  Bulk traffic rides SWDGE
    deep-queued, with a few mid-stream chunks on the HWDGE queues so the PE
    has work before the SWDGE bulk lands.
"""

import sys

if "/opt/trn_rl_repo" not in sys.path:
    sys.path.insert(0, "/opt/trn_rl_repo")

import numpy as np

import concourse.bass as bass
import concourse.tile as tile
from concourse import bacc, mybir
from concourse.bass_utils import run_bass_kernel_spmd
from concourse.masks import make_identity

F32 = mybir.dt.float32
BF16 = mybir.dt.bfloat16
I16 = mybir.dt.int16
ACT_F = mybir.ActivationFunctionType
ALU = mybir.AluOpType

B, C, H, W = 8, 64, 256, 256
N = H * W          # 65536
HALF = N // 2      # 32768 columns per partition-half
CH = 2048          # bulk chunk columns (stacked layout) -> 512 KiB bf16
NCHUNK = HALF // CH  # 16
RPC = N // CH      # DRAM rows per channel in [rows, CH] view (16)
NPRE = 5           # input chunks issued ahead (SWDGE descriptor window)
SUB = 128          # transpose subblock columns
TB = 4             # transposes per PSUM batch ([128, 512])
P2CH = 512         # pass-2 chunk columns (one PSUM bank)
OG = 8             # pass-2 chunks per output stage ([128, 4096] -> 1 MiB)
NQ = 4             # SWDGE queues


def build(sim_safe=False):
    nc = bacc.Bacc(None, target_bir_lowering=False)
    x_d = nc.dram_tensor("x", [C, N], BF16, kind="ExternalInput")
    g_d = nc.dram_tensor("gamma", [1, 1], F32, kind="ExternalInput")
    out_d = nc.dram_tensor("out", [C, N], BF16, kind="ExternalOutput")

    # 3D views ordered (h, c, n): stream order matches the stacked SBUF
    # layout [p = h*64+c, n]; one DMA covers both partition halves.
    x_v = x_d.ap().rearrange("c (h n) -> h c n", h=2)
    out_v = out_d.ap().rearrange("c (h n) -> h c n", h=2)

    with tile.TileContext(nc) as tc, \
         tc.tile_pool(name="constp", bufs=1) as constp, \
         tc.tile_pool(name="smalls", bufs=2) as smalls:
        # resident x (bf16), stacked layout [128, HALF]
        xb_sb = constp.tile([128, HALF], BF16)

        def in_dma(k):
            nc.gpsimd.dma_start(
                out=xb_sb[:, k * CH : (k + 1) * CH],
                in_=x_v[:, :, k * CH : (k + 1) * CH],
            )

        # First chunk load goes out before anything else; the remaining
        # upfront loads are spaced with ~4us gpsimd delay memsets so chunk
        # completions stagger (the SWDGE queue serves all queued transfers
        # round-robin, so same-time issues complete together and leave the
        # PE bursty).
        in_dma(0)

        # ---------------- constants / persistent tiles ----------------
        ident128 = constp.tile([128, 128], BF16)
        make_identity(nc, ident128)
        ident64f = constp.tile([64, 64], F32)
        make_identity(nc, ident64f)
        ones_row = constp.tile([1, 128], F32)
        nc.gpsimd.memset(ones_row, 1.0)

        delay_sb = constp.tile([128, 1536], BF16)
        for k in range(1, NPRE):
            nc.gpsimd.memset(delay_sb, 0.0)
            in_dma(k)

        g_sb = constp.tile([1, 1], F32)
        nc.sync.dma_start(out=g_sb, in_=g_d.ap())

        # [1,1] probe target for DMA pacing reads
        probe_sb = constp.tile([1, 1], BF16)

        # staging ring for transposed tiles: TB groups of 129 columns,
        # group = [xT(128) | 1]; col 128 preset 1.0 (never overwritten)
        NRING = 3
        T_st = []
        for i in range(NRING):
            t = constp.tile([128, 129 * TB], BF16, name=f"T_st{i}")
            T_st.append(t)
            nc.gpsimd.memset(
                t.rearrange("p (g w) -> p g w", w=129)[:, :, 128:129], 1.0
            )

        # block-diagonal [[A'^T, 0], [0, A'^T]] lhsT for pass 2
        AT2 = constp.tile([128, 128], BF16)
        nc.gpsimd.memset(AT2, 0.0)
        negb = constp.tile([128, 1], F32)
        g_bcast = constp.tile([128, 1], F32)
        u2_bf = constp.tile([128, 1], BF16)

        with (
            tc.tile_pool(name="psG", bufs=1, space="PSUM") as psG,
            tc.tile_pool(name="psT", bufs=2, space="PSUM") as psT,
            tc.tile_pool(name="psS", bufs=2, space="PSUM") as psS,
        ):
            # fused Gram accumulator: [0:64,0:64]+[64:128,64:128] = raw Gram
            # blocks per half; col 128 = per-stacked-channel sums
            G2 = psG.tile([128, 129], F32, tag="g2")

            # PE warmup: absorb gpsimd const deps into the PE clock.
            warm_ps = psS.tile([128, 128], BF16, tag="small")
            nc.tensor.matmul(warm_ps, ident128, ident128, is_transpose=True)
            # preload exp activation table early (off the critical path)
            exp_scr = smalls.tile([1, 1], F32, tag="escr")
            nc.scalar.activation(exp_scr, ones_row[0:1, 0:1], ACT_F.Exp)

            # gamma broadcast to all 128 partitions (K=1 matmul trick)
            gb_ps = psS.tile([128, 1], F32, tag="small")
            nc.tensor.matmul(gb_ps, ones_row, g_sb, start=True, stop=True)
            nc.vector.tensor_copy(g_bcast, gb_ps)

            # ---------------- phase 1: load + transpose + Gram ----------------
            # Per chunk: CH/SUB transposes in batches of TB; each batch: TB
            # transposes -> one strided PSUM->staging copy (DVE/ACT
            # alternating) -> TB [128,129] Gram matmuls.  Batches are
            # software-pipelined: batch b's Gram matmuls are emitted after
            # batch b+1's transposes so the PE never waits on the staging
            # copy.
            total_batches = NCHUNK * (CH // (SUB * TB))  # 16 * 4 = 64
            nbpc = CH // (SUB * TB)                      # batches per chunk

            def emit_gram_mms(b):
                st = T_st[b % NRING]
                st_v = st.rearrange("p (g w) -> p g w", w=129)
                for g in range(TB):
                    nc.tensor.matmul(
                        G2,
                        st_v[:, g, 0:128],
                        st_v[:, g, 0:129],
                        start=(b == 0 and g == 0),
                        stop=(b == total_batches - 1 and g == TB - 1),
                        skip_group_check=True,
                    )

            for k in range(NCHUNK):
                xslice = xb_sb[:, k * CH : (k + 1) * CH]
                # DGE pacing: before issuing chunk k's load, make the (idle)
                # gpsimd engine block until chunk k-NPRE has fully landed.
                # This bounds the SWDGE descriptor window so chunk
                # completions stagger instead of all arriving at the end of
                # the whole input phase (measured pathology).
                if k >= NPRE:
                    nc.gpsimd.tensor_copy(
                        probe_sb,
                        xb_sb[0:1, (k - NPRE) * CH : (k - NPRE) * CH + 1],
                    )
                    in_dma(k)
                for bb in range(nbpc):
                    b = k * nbpc + bb
                    ph = psT.tile([128, SUB * TB], BF16, tag="psTh")
                    for jj in range(TB):
                        src = slice((bb * TB + jj) * SUB, (bb * TB + jj + 1) * SUB)
                        nc.tensor.matmul(
                            ph[:, jj * SUB : (jj + 1) * SUB],
                            xslice[:, src],
                            ident128,
                            is_transpose=True,
                        )
                    st = T_st[b % NRING]
                    st_v = st.rearrange("p (g w) -> p g w", w=129)
                    ph_v = ph.rearrange("p (g w) -> p g w", w=SUB)
                    if b % 2 == 0:
                        nc.vector.tensor_copy(st_v[:, :, 0:128], ph_v)
                    else:
                        nc.scalar.activation(st_v[:, :, 0:128], ph_v, ACT_F.Copy)
                    if b > 0:
                        emit_gram_mms(b - 1)
            emit_gram_mms(total_batches - 1)

            # ------------- phase 1.5: energy, softmax, A', bias -----------
            Gs = smalls.tile([128, 129], F32, tag="gs")
            nc.scalar.activation(Gs, G2, ACT_F.Copy)
            # bring the half-1 diag block + sums down to partitions 0:64
            Gtmp = smalls.tile([64, 65], F32, tag="gtmp")
            nc.sync.dma_start(out=Gtmp, in_=Gs[64:128, 64:129])
            Gsum = smalls.tile([64, 64], F32, tag="gsum")
            nc.vector.tensor_add(Gsum, Gs[0:64, 0:64], Gtmp[:, 0:64])
            s_col = smalls.tile([64, 1], F32, tag="scol")
            nc.vector.tensor_add(s_col, Gs[0:64, 128:129], Gtmp[:, 64:65])
            u_col = smalls.tile([64, 1], F32, tag="ucol")
            nc.vector.tensor_scalar_mul(u_col, s_col, 1.0 / N)

            # outer product uu^T via [64,1] -> [1,64] transpose + K=1 matmul
            urow_ps = psS.tile([1, 64], F32, tag="small")
            nc.tensor.matmul(urow_ps, u_col, ident64f, is_transpose=True)
            urow = smalls.tile([1, 64], F32, tag="urow")
            nc.vector.tensor_copy(urow, urow_ps)
            uuT_ps = psS.tile([64, 64], F32, tag="small")
            nc.tensor.matmul(uuT_ps, urow, urow, start=True, stop=True)

            # E = Gsum/N - uu^T
            E_sb = smalls.tile([64, 64], F32, tag="esb")
            nc.vector.tensor_scalar_mul(E_sb, Gsum, 1.0 / N)
            nc.vector.tensor_sub(E_sb, E_sb, uuT_ps)

            # row softmax; fold gamma into A'
            negm = smalls.tile([64, 1], F32, tag="negm")
            nc.vector.tensor_reduce(
                negm, E_sb, axis=mybir.AxisListType.X, op=ALU.max, negate=True
            )
            P_sb = smalls.tile([64, 64], F32, tag="psb")
            nc.scalar.activation(P_sb, E_sb, ACT_F.Exp, bias=negm, scale=1.0)
            z = smalls.tile([64, 1], F32, tag="z")
            nc.vector.reduce_sum(z, P_sb, axis=mybir.AxisListType.X)
            rz = smalls.tile([64, 1], F32, tag="rz")
            nc.vector.reciprocal(rz, z)
            rg = smalls.tile([64, 1], F32, tag="rg")
            nc.vector.tensor_mul(rg, rz, g_bcast[0:64, :])
            A2 = smalls.tile([64, 128], F32, tag="a2")
            nc.vector.tensor_scalar_mul(A2[:, 0:64], P_sb, rg)
            nc.vector.tensor_copy(A2[:, 64:128], A2[:, 0:64])

            # AT2 block-diag: transpose [A'|A'] -> [128,64] stacked, then
            # bf16-cast the two halves into the diagonal blocks
            AT_ps = psS.tile([128, 64], F32, tag="small")
            nc.tensor.matmul(AT_ps, A2, ident64f, is_transpose=True)
            nc.vector.tensor_copy(AT2[0:64, 0:64], AT_ps[0:64, :])
            nc.scalar.activation(AT2[64:128, 64:128], AT_ps[64:128, :], ACT_F.Copy)

            # bias: negb = -(A' @ u) on all 128 partitions
            nc.vector.tensor_copy(u2_bf[0:64, :], u_col)
            nc.sync.dma_start(out=u2_bf[64:128, :], in_=u2_bf[0:64, :])
            b1_ps = psS.tile([128, 1], F32, tag="small")
            nc.tensor.matmul(b1_ps, AT2, u2_bf, start=True, stop=True)
            nc.vector.tensor_scalar_mul(negb, b1_ps, -1.0)
            # touch negb on ACT too so the epilogue's ACT instrs have their
            # wait absorbed off the critical path
            scr_a = smalls.tile([128, 1], F32, tag="scra")
            nc.scalar.activation(scr_a, negb, ACT_F.Copy)

        # ---------------- phase 2: out = A'@x + negb ----------------
        with (
            tc.tile_pool(name="ps2", bufs=8, space="PSUM") as ps2,
            tc.tile_pool(name="ostage", bufs=8) as ostage,
        ):
            n_ch = HALF // P2CH  # 64
            stage = None
            for p in range(n_ch):
                if p % OG == 0:
                    stage = ostage.tile([128, OG * P2CH], BF16, tag="ost")
                pso = ps2.tile([128, P2CH], F32, tag="pso")
                cols = slice(p * P2CH, (p + 1) * P2CH)
                nc.tensor.matmul(
                    pso, AT2, xb_sb[:, cols], start=True, stop=True,
                )
                dst = stage[:, (p % OG) * P2CH : (p % OG + 1) * P2CH]
                if p % 2 == 0:
                    nc.vector.tensor_scalar_add(dst, pso, negb)
                else:
                    # CoreSim lacks Prelu; Relu with bias has identical cost
                    # and serves for sim-timing runs.
                    f = ACT_F.Relu if sim_safe else ACT_F.Prelu
                    nc.scalar.activation(
                        dst, pso, f, bias=negb, scale=1.0, alpha=1.0
                    )
                if p % OG == OG - 1:
                    q = p // OG
                    dst = out_v[:, :, q * OG * P2CH : (q + 1) * OG * P2CH]
                    nc.gpsimd.dma_start(out=dst, in_=stage)

    nc.finalize()
    return nc


_CACHED = None


def _get_nc():
    global _CACHED
    if _CACHED is None:
        _CACHED = build()
    return _CACHED


def _to_bf16(a):
    import ml_dtypes

    return np.asarray(a).astype(ml_dtypes.bfloat16)


def _in_maps(x: np.ndarray, gamma: np.ndarray) -> list:
    xb = _to_bf16(np.ascontiguousarray(x)).reshape(B, C, N)
    g = np.asarray(gamma, dtype=np.float32).reshape(1, 1)
    return [{"x": xb[i], "gamma": g} for i in range(B)]


def kernel(x: np.ndarray, gamma: np.ndarray) -> np.ndarray:
    assert x.shape == (B, C, H, W), x.shape
    nc = _get_nc()
    res = run_bass_kernel_spmd(nc, _in_maps(x, gamma), core_ids=list(range(B)))
    out = np.stack([res.results[i]["out"] for i in range(B)])
    return out.reshape(B, C, H, W).astype(np.float32)


if __name__ == "__main__":
    rng = np.random.default_rng(0)
    x = rng.standard_normal((B, C, H, W), dtype=np.float32)
    gamma = rng.standard_normal((1,), dtype=np.float32)
    y = kernel(x, gamma)
    print("ran ok", y.shape, y.dtype)
